# revision 9
# baseline (speedup 1.0000x reference)
"""DANet-style channel attention kernel for Trainium2 (8 NeuronCores).

Problem (hardcoded): B=16, C=256, H=W=128 (N=HW=16384), fp32.
  q = Wq@Q+bq; k = Wk@K+bk; v = Wv@X+bv          (1x1 convs, per batch elem)
  energy = q @ k^T            [C,C]
  attn   = softmax(rowmax(energy) - energy)       (== softmax(-energy))
  out    = attn @ v           [C,N]

Key algebraic refactor vs the v1 kernel: the v-projection and the
attention-apply are fused into ONE pass over X:

  out = attn @ (Wv X + bv 1^T) = (attn Wv) @ X + (attn bv) 1^T
      =      M @ X + c 1^T

M = attn@Wv is a tiny [256,256] GEMM and c = attn@bv a [256] vector, both
computed right after the softmax; the big [256,256]x[256,16384] pass over X
then happens once instead of twice (v-projection + attn@v).  This removes
one of the five full GEMM passes per batch element (~20% of PE work) at
identical HBM traffic.

Sharding: data-parallel over batch; 2 batch elements per core, 8 cores.

Per-core structure (per batch element; phases of adjacent elements overlap):
  A: stream q,k,x in 1 MiB chunks (q->SP ring, k->ACT ring, x alternating);
     per 512-px compute chunk produce qT/kT tiles [n128 x f256] directly in
     transposed layout (the input tile is the PE stationary operand, W^T the
     moving operand -> no transposes anywhere), add biases via one DVE
     tensor_add per chunk, and accumulate the full energy [256,256] in one
     persistent PSUM bank across all 16384 pixels.  X is converted fp32->fp16
     on the scalar engine into 8 resident SBUF tiles (8 MiB total; split into
     8 tiles so element b+1's writes overlap element b's phase-C reads).
  B: rowmin via DVE reduce(min); P = Exp(-energy + rowmin) on ACT with fused
     row-sum (accum_out); attn = P * (1/rowsum) scaled in-place on DVE;
     c = attn@bv via one DVE tensor_tensor_reduce; PE-transpose of the four
     128x128 attn blocks -> PT; MT = Wv^T-stationary matmul against PT
     (M transposed, [c,e]) -> fp16.
  C: out = MT.T @ X + c with N=512 fp16 matmuls from resident X,
     double-buffered PSUM; PSUM->SBUF copy + bias alternates between ACT
     (activation bias) and DVE (tensor_scalar_add); 2 MiB stores on SWDGE.

Matmuls run as float32r (FP22: fp32 bytes, truncated mantissa; full PE rate
at free-dim >= 256); fp16 for the M@X stage.

PSUM budget (8 banks): qt 2 + kt 2 + energy/PT 1 + MT 1 + out 2.

Walrus constraint handled here: a fused-LDW (4-byte dtype) matmul carries at
most ONE semaphore wait, and bass'es legalization for that lives in Bacc
(generate_event_semaphores), so the module is built with bacc.Bacc() and
finalized before execution.
"""

import numpy as np

B_FULL = 16
N_CORES = 8
B2 = B_FULL // N_CORES  # batch elems per core
C = 256
N = 16384  # H*W
CH_DA = 1024  # phase-A DMA chunk (pixels) -> 1 MiB per load
CH_DO = 2048  # phase-C store chunk (pixels) -> 2 MiB per store
CH_A = 512    # phase-A compute chunk (pixels)
CH_X = 2048   # resident-X tile granularity (pixels) -> 1 MiB fp16 per tile
N_XT = N // CH_X

_CACHE = {}


def _build(loop_k=None):
    import contextlib

    import concourse.bass as bass
    import concourse.tile as tile
    from concourse import bacc, mybir

    f32 = mybir.dt.float32
    f32r = mybir.dt.float32r
    f16 = mybir.dt.float16
    AF = mybir.ActivationFunctionType
    AX = mybir.AxisListType
    OP = mybir.AluOpType

    nc = bacc.Bacc()

    q_in = nc.declare_dram_parameter("q_in", [B2, C, N], f32r, isOutput=False)
    k_in = nc.declare_dram_parameter("k_in", [B2, C, N], f32r, isOutput=False)
    x_in = nc.declare_dram_parameter("x_in", [B2, C, N], f32r, isOutput=False)
    wqt_d = nc.declare_dram_parameter("wqt", [C, C], f32r, isOutput=False)
    wkt_d = nc.declare_dram_parameter("wkt", [C, C], f32r, isOutput=False)
    wvn_d = nc.declare_dram_parameter("wvn", [C, C], f32r, isOutput=False)
    bqb_d = nc.declare_dram_parameter("bqb", [128, 4, 256], f32, isOutput=False)
    bkb_d = nc.declare_dram_parameter("bkb", [128, 4, 256], f32, isOutput=False)
    bvr_d = nc.declare_dram_parameter("bvr", [128, 256], f32, isOutput=False)
    id_d = nc.declare_dram_parameter("ident", [128, 128], f32, isOutput=False)
    out_d = nc.declare_dram_parameter("out", [B2, C, N], f32, isOutput=True)

    with tile.TileContext(nc) as tc:
        with (
            tc.tile_pool(name="const", bufs=1) as const,
            tc.tile_pool(name="xres", bufs=1) as xres,
            tc.tile_pool(name="qkc", bufs=2) as qkc,
            tc.tile_pool(name="xc_p", bufs=2) as xc_p,
            tc.tile_pool(name="tsb", bufs=3) as tsb,
            tc.tile_pool(name="osb", bufs=2) as osb,
            tc.tile_pool(name="smax", bufs=2) as smax,
            tc.tile_pool(name="ps_qt", bufs=1, space="PSUM") as ps_qt,
            tc.tile_pool(name="ps_kt", bufs=1, space="PSUM") as ps_kt,
            tc.tile_pool(name="ps_e", bufs=1, space="PSUM") as ps_e,
            tc.tile_pool(name="ps_m", bufs=1, space="PSUM") as ps_m,
            tc.tile_pool(name="ps_o", bufs=2, space="PSUM") as ps_o,
        ):
            # ---- constants ----
            wqt = const.tile([128, 2, C], f32r)
            wkt = const.tile([128, 2, C], f32r)
            wvn = const.tile([128, 2, C], f32r)
            for w_sb, w_d in ((wqt, wqt_d), (wkt, wkt_d), (wvn, wvn_d)):
                nc.sync.dma_start(
                    out=w_sb[:, :, :],
                    in_=w_d[:, :].rearrange("(t p) f -> p t f", p=128))
            bqb = const.tile([128, 4, 256], f32)
            bkb = const.tile([128, 4, 256], f32)
            bvr = const.tile([128, 256], f32)
            ident = const.tile([128, 128], f32)
            nc.sync.dma_start(out=bqb[:, :, :], in_=bqb_d[:, :, :])
            nc.sync.dma_start(out=bkb[:, :, :], in_=bkb_d[:, :, :])
            nc.sync.dma_start(out=bvr[:, :], in_=bvr_d[:, :])
            nc.sync.dma_start(out=ident[:, :], in_=id_d[:, :])

            n_sub_a = CH_A // 128
            loop_cm = (
                tc.For_i(0, loop_k) if loop_k is not None
                else contextlib.nullcontext()
            )
            with loop_cm:
              for b in range(B2):
                # == phase A: stream q,k,x; energy accum + X (fp16 resident)
                e_ps = ps_e.tile([128, 2, 256], f32, tag="e")
                xs = [xres.tile([128, 2, CH_X], f16, tag=f"x{j}",
                                name=f"x16_{j}")
                      for j in range(N_XT)]
                for cd in range(N // CH_DA):
                  qc = qkc.tile([128, 2, CH_DA], f32r, tag="qc")
                  kc = qkc.tile([128, 2, CH_DA], f32r, tag="kc")
                  xc = xc_p.tile([128, 2, CH_DA], f32r, tag="xc")
                  base = cd * CH_DA
                  nc.sync.dma_start(
                      out=qc[:, :, :],
                      in_=q_in[b, :, base:base + CH_DA].rearrange(
                          "(t p) n -> p t n", p=128))
                  nc.scalar.dma_start(
                      out=kc[:, :, :],
                      in_=k_in[b, :, base:base + CH_DA].rearrange(
                          "(t p) n -> p t n", p=128))
                  x_eng = nc.sync if cd % 2 == 0 else nc.scalar
                  x_eng.dma_start(
                      out=xc[:, :, :],
                      in_=x_in[b, :, base:base + CH_DA].rearrange(
                          "(t p) n -> p t n", p=128))
                  # X -> fp16 resident (scalar engine)
                  xt = xs[base // CH_X]
                  xo = base % CH_X
                  nc.scalar.activation(
                      out=xt[:, :, xo:xo + CH_DA], in_=xc[:, :, :],
                      func=AF.Identity)
                  for cc in range(CH_DA // CH_A):
                    ci = cd * (CH_DA // CH_A) + cc
                    co = cc * CH_A  # offset within the DMA chunk
                    qt_sb = tsb.tile([128, n_sub_a, 256], f32r, tag="qt_sb")
                    kt_sb = tsb.tile([128, n_sub_a, 256], f32r, tag="kt_sb")
                    qt_ps = ps_qt.tile([128, n_sub_a, 256], f32)
                    kt_ps = ps_kt.tile([128, n_sub_a, 256], f32)
                    for ns in range(n_sub_a):
                        for ct in range(2):
                            nc.tensor.matmul(
                                qt_ps[:, ns, :],
                                lhsT=qc[:, ct, co + ns * 128:
                                        co + (ns + 1) * 128],
                                rhs=wqt[:, ct, :],
                                start=(ct == 0 and ns % 2 == 0),
                                stop=(ct == 1),
                                skip_group_check=True)
                        for ct in range(2):
                            nc.tensor.matmul(
                                kt_ps[:, ns, :],
                                lhsT=kc[:, ct, co + ns * 128:
                                        co + (ns + 1) * 128],
                                rhs=wkt[:, ct, :],
                                start=(ct == 0 and ns % 2 == 0),
                                stop=(ct == 1),
                                skip_group_check=True)
                    # single bias add (broadcast along partitions) + to SBUF
                    nc.vector.tensor_add(
                        qt_sb[:, :, :], qt_ps[:, :, :], bqb[:, :, :])
                    nc.vector.tensor_add(
                        kt_sb[:, :, :], kt_ps[:, :, :], bkb[:, :, :])
                    # energy += qT^T @ kT
                    for ns in range(n_sub_a):
                        for cm in range(2):
                            nc.tensor.matmul(
                                e_ps[:, cm, :],
                                lhsT=qt_sb[:, ns,
                                           cm * 128:(cm + 1) * 128],
                                rhs=kt_sb[:, ns, :],
                                start=(ci == 0 and ns == 0 and cm == 0),
                                stop=(ci == N // CH_A - 1
                                      and ns == n_sub_a - 1),
                                skip_group_check=True)

                # ================= phase B: negated softmax ==============
                rmin = smax.tile([128, 2], f32, tag="rmin")
                rsum = smax.tile([128, 2], f32, tag="rsum")
                rinv = smax.tile([128, 2], f32, tag="rinv")
                p_sb = smax.tile([128, 2, 256], f32, tag="p_sb")
                cv_sb = smax.tile([128, 2], f32, tag="cv_sb")
                cv_scr = smax.tile([128, 256], f32, tag="cv_scr")
                for cm in range(2):
                    nc.vector.tensor_reduce(
                        out=rmin[:, cm:cm + 1], in_=e_ps[:, cm, :],
                        axis=AX.X, op=OP.min)
                    # P = exp(-energy + rowmin), rowsum fused
                    nc.scalar.activation(
                        out=p_sb[:, cm, :], in_=e_ps[:, cm, :], func=AF.Exp,
                        bias=rmin[:, cm:cm + 1], scale=-1.0,
                        accum_out=rsum[:, cm:cm + 1])
                nc.vector.reciprocal(rinv[:, :], rsum[:, :])
                # attn = P / rowsum (in-place scale), c = attn @ bv
                for cm in range(2):
                    nc.vector.tensor_scalar_mul(
                        p_sb[:, cm, :], p_sb[:, cm, :], rinv[:, cm:cm + 1])
                    nc.vector.tensor_tensor_reduce(
                        out=cv_scr[:, :], in0=p_sb[:, cm, :], in1=bvr[:, :],
                        scale=1.0, scalar=0.0, op0=OP.mult, op1=OP.add,
                        accum_out=cv_sb[:, cm:cm + 1])
                # PT[f, e] via PE transpose of the four 128x128 attn blocks
                pt_ps = ps_e.tile([128, 2, 256], f32, tag="e")
                pt_sb = smax.tile([128, 2, 256], f32r, tag="pt_sb")
                for dt in range(2):
                    for cm in range(2):
                        nc.tensor.transpose(
                            out=pt_ps[:, dt, cm * 128:(cm + 1) * 128],
                            in_=p_sb[:, cm, dt * 128:(dt + 1) * 128],
                            identity=ident[:, :])
                nc.vector.tensor_copy(pt_sb[:, :, :], pt_ps[:, :, :])
                # MT[c, e] = sum_f Wv[f, c] * PT[f, e]   (= (attn@Wv)^T)
                mt_ps = ps_m.tile([128, 2, 256], f32)
                mt_sb = smax.tile([128, 2, 256], f16, tag="mt_sb")
                for cb in range(2):
                    for fb in range(2):
                        nc.tensor.matmul(
                            mt_ps[:, cb, :],
                            lhsT=wvn[:, fb, cb * 128:(cb + 1) * 128],
                            rhs=pt_sb[:, fb, :],
                            start=(fb == 0), stop=(fb == 1))
                nc.vector.tensor_copy(mt_sb[:, :, :], mt_ps[:, :, :])

                # ========== phase C: out = MT.T @ X + c, streamed out ====
                for cd in range(N // CH_DO):
                    off = cd * CH_DO
                    o_sb = osb.tile([128, 2, CH_DO], f32)
                    for sub in range(CH_DO // CH_A):
                        so = off + sub * CH_A
                        xt = xs[so // CH_X]
                        xo = so % CH_X
                        for eb in range(2):
                            o_ps = ps_o.tile([128, CH_A], f32)
                            for cb in range(2):
                                nc.tensor.matmul(
                                    o_ps[:, :],
                                    lhsT=mt_sb[:, cb,
                                               eb * 128:(eb + 1) * 128],
                                    rhs=xt[:, cb, xo:xo + CH_A],
                                    start=(cb == 0), stop=(cb == 1))
                            dst = o_sb[:, eb,
                                       sub * CH_A:(sub + 1) * CH_A]
                            if eb == 0:
                                nc.scalar.activation(
                                    out=dst, in_=o_ps[:, :],
                                    func=AF.Identity,
                                    bias=cv_sb[:, eb:eb + 1])
                            else:
                                nc.vector.tensor_scalar_add(
                                    dst, o_ps[:, :], cv_sb[:, eb:eb + 1])
                    nc.gpsimd.dma_start(
                        out=out_d[b, :, off:off + CH_DO].rearrange(
                            "(t p) n -> p t n", p=128),
                        in_=o_sb[:, :, :])
    if not nc.is_finalized():
        nc.finalize()
    return nc


def _consts(Wq, bq, Wk, bk, Wv, bv):
    return {
        "wqt": np.ascontiguousarray(Wq.T),
        "wkt": np.ascontiguousarray(Wk.T),
        "wvn": np.ascontiguousarray(Wv),
        "bqb": np.ascontiguousarray(
            np.broadcast_to(bq[None, None, :], (128, 4, 256))),
        "bkb": np.ascontiguousarray(
            np.broadcast_to(bk[None, None, :], (128, 4, 256))),
        "bvr": np.ascontiguousarray(
            np.broadcast_to(bv[None, :], (128, 256))),
        "ident": np.eye(128, dtype=np.float32),
    }


def kernel(query, key, x, Wq, bq, Wk, bk, Wv, bv):
    from concourse.bass_utils import run_bass_kernel_spmd

    query = np.ascontiguousarray(np.asarray(query, dtype=np.float32))
    key = np.ascontiguousarray(np.asarray(key, dtype=np.float32))
    x = np.ascontiguousarray(np.asarray(x, dtype=np.float32))
    Wq = np.asarray(Wq, dtype=np.float32)
    bq = np.asarray(bq, dtype=np.float32)
    Wk = np.asarray(Wk, dtype=np.float32)
    bk = np.asarray(bk, dtype=np.float32)
    Wv = np.asarray(Wv, dtype=np.float32)
    bv = np.asarray(bv, dtype=np.float32)

    B, Cc, H, W = query.shape
    assert (B, Cc, H * W) == (B_FULL, C, N)

    if "nc" not in _CACHE:
        _CACHE["nc"] = _build()
    nc = _CACHE["nc"]

    consts = _consts(Wq, bq, Wk, bk, Wv, bv)
    in_maps = []
    for i in range(N_CORES):
        sl = slice(i * B2, (i + 1) * B2)
        in_maps.append({
            "q_in": query[sl].reshape(B2, C, N),
            "k_in": key[sl].reshape(B2, C, N),
            "x_in": x[sl].reshape(B2, C, N),
            **consts,
        })

    res = run_bass_kernel_spmd(nc, in_maps, list(range(N_CORES)))
    out = np.concatenate([res.results[i]["out"] for i in range(N_CORES)], axis=0)
    return out.reshape(B_FULL, C, H, W).astype(np.float32)
